# revision 8
# baseline (speedup 1.0000x reference)
"""AAL positional embedding lookup on 8 TRN2 NeuronCores.

Per core (data-parallel over B, 2 batches = 8192 points per core):
  1. Centers loaded as ONE contiguous [128, 192] DMA (point j lives at
     partition j//64, col j%64 -- "PC" layout); the affine transform +
     round-half-even (1.5*2^23 magic-add, bit-exact with jnp.round) runs
     on stride-3 views of that tile.  Bounds mask, clamp, linear voxel
     index, 256-voxel block id + in-block offset (all exact in f32).
  2. Block ids -> SWDGE wrapped-16 int16 layout via a masked PE matmul
     (selw/maskw); 8x gpsimd dma_gather, each fetching 1024 points'
     512B bf16 atlas blocks (atlas pre-cast to bf16 on host; region ids
     <= 116 are exact in bf16).  Gather list position i = 128*c + p so
     each point's block lands at its own (partition, col).  gpsimd does
     NOTHING else: its ~9.5ns/point descriptor generation is the
     critical path, everything else hides under it.
  3. Extraction off gpsimd: the one-hot offset mask for ALL 64 cols is
     precomputed in one DVE op (emask_all, hidden under gather 0); per
     slice only a dense bf16 multiply (DVE 2x mode) + reduce remain.
  4. One-hot: psB[k, t] = broadcast of region_t over 117 partitions via
     a masked matmul (region*ident, all-ones lhsT); is_equal against a
     per-partition iota -> bf16 onehot for 8 chunks at once.
  5. Embedding rows = onehot.T @ table (bf16 matmul, N=512/256);
     PSUM->SBUF bf16 copies all on ACT; out DMAs all on the sync HWDGE
     queue.  Output chunk c = out rows {64p + c} (row stride 64),
     1536B/row.  Host converts bf16 -> f32 (values are exactly bf16).
"""

import numpy as np

B, N = 16, 4096
D, H, W = 182, 218, 182
EMBED = 768
REGION_MAX = 116
NCORES = 8
PTS = (B // NCORES) * N          # 8192 points per core
COLS = PTS // 128                # 64
NSLICE = 8
SCOLS = COLS // NSLICE           # 8 cols per slice
SPTS = SCOLS * 128               # 1024 points per slice
BLK = 256                        # atlas block: 256 bf16 = 512B
NBLK = (D * H * W + BLK - 1) // BLK
MAGIC = 12582912.0               # 1.5 * 2^23: RNE rounding trick
FLOOR_C = 0.498046875            # 255/512: floor via round-to-nearest

_cache = {}


def _build(m34):
    import concourse.bacc as bacc
    import concourse.mybir as mybir
    import concourse.tile as tile

    dt = mybir.dt
    Alu = mybir.AluOpType

    is_ident = (
        np.array_equal(m34[:, :3], np.eye(3, dtype=np.float32))
        and np.all(m34[:, 3] == 0.0)
    )

    nc = bacc.Bacc("TRN2", target_bir_lowering=False)

    centers = nc.declare_dram_parameter("centers", [PTS, 3], dt.float32, isOutput=False)
    atlas = nc.declare_dram_parameter("atlas", [NBLK, BLK], dt.bfloat16, isOutput=False)
    table = nc.declare_dram_parameter("table", [REGION_MAX + 1, EMBED], dt.bfloat16, isOutput=False)
    iotar_d = nc.declare_dram_parameter("iotar", [128, 1], dt.float32, isOutput=False)
    ident_d = nc.declare_dram_parameter("ident", [128, 128], dt.float32, isOutput=False)
    ones_d = nc.declare_dram_parameter("ones", [128, REGION_MAX + 1], dt.bfloat16, isOutput=False)
    selw_d = nc.declare_dram_parameter("selw", [128, 128], dt.float32, isOutput=False)
    maskw_d = nc.declare_dram_parameter("maskw", [128, COLS * 8], dt.float32, isOutput=False)
    iota256_d = nc.declare_dram_parameter("iota256", [128, BLK], dt.bfloat16, isOutput=False)
    out_d = nc.declare_dram_parameter("out", [PTS, EMBED], dt.bfloat16, isOutput=True)

    # out rows {64p + c} for chunk c: [128, 64*768] view, col-slice per chunk
    out_v = out_d[:, :].rearrange("(p c) e -> p (c e)", p=128)

    with tile.TileContext(nc) as tc:
        with (
            tc.tile_pool(name="const", bufs=1) as cpool,
            tc.tile_pool(name="work", bufs=1) as wpool,
            tc.tile_pool(name="blocks", bufs=8) as bpool,
            tc.tile_pool(name="sl", bufs=2) as spool,
            tc.tile_pool(name="oh", bufs=2) as ohpool,
            tc.tile_pool(name="osb", bufs=4) as opool,
            tc.tile_pool(name="psX", bufs=2, space="PSUM") as psXp,
            tc.tile_pool(name="psO", bufs=2, space="PSUM") as psOp,
        ):
            # ---- centers FIRST (critical path): one contiguous DMA ----
            cent = wpool.tile([128, COLS * 3], dt.float32, tag="cent")
            nc.sync.dma_start(
                cent[:], centers[:, :].rearrange("(p c) k -> p (c k)", p=128)
            )
            # strided views: coord k = cent[:, k::3]
            cent3 = cent[:].rearrange("p (c k) -> p k c", k=3)
            coord = [
                cent3[:, k : k + 1, :].rearrange("p one c -> p (one c)")
                for k in range(3)
            ]

            # ---- constants (selw/maskw early on sync; bulk on scalar) ----
            selw = cpool.tile([128, 128], dt.float32)
            nc.sync.dma_start(selw[:], selw_d[:, :])
            maskw = cpool.tile([128, COLS * 8], dt.float32)
            nc.sync.dma_start(maskw[:], maskw_d[:, :])
            iotar = cpool.tile([128, 1], dt.float32)
            nc.scalar.dma_start(iotar[:], iotar_d[:, :])
            iota256 = cpool.tile([128, BLK], dt.bfloat16)
            nc.scalar.dma_start(iota256[:], iota256_d[:, :])
            ident_f = cpool.tile([128, 128], dt.float32)
            nc.scalar.dma_start(ident_f[:], ident_d[:, :])
            ones_bf = cpool.tile([128, REGION_MAX + 1], dt.bfloat16)
            nc.scalar.dma_start(ones_bf[:], ones_d[:, :])
            table_bf = cpool.tile([REGION_MAX + 1, EMBED], dt.bfloat16)
            nc.scalar.dma_start(table_bf[:], table[:, :])

            # ---- pointwise: transform + round (RNE) ----
            r = []
            for k in range(3):
                rk = wpool.tile([128, COLS], dt.float32, tag=f"r{k}")
                if is_ident:
                    nc.vector.tensor_scalar(
                        rk[:], coord[k], MAGIC, MAGIC, op0=Alu.add, op1=Alu.subtract
                    )
                else:
                    t0 = wpool.tile([128, COLS], dt.float32, tag="t0")
                    nc.vector.tensor_scalar_mul(t0[:], coord[0], float(m34[k, 0]))
                    nc.vector.scalar_tensor_tensor(
                        t0[:], coord[1], float(m34[k, 1]), t0[:],
                        op0=Alu.mult, op1=Alu.add,
                    )
                    nc.vector.scalar_tensor_tensor(
                        t0[:], coord[2], float(m34[k, 2]), t0[:],
                        op0=Alu.mult, op1=Alu.add,
                    )
                    nc.vector.tensor_scalar_add(t0[:], t0[:], float(m34[k, 3]))
                    nc.vector.tensor_scalar(
                        rk[:], t0[:], MAGIC, MAGIC, op0=Alu.add, op1=Alu.subtract
                    )
                r.append(rk)

            # ---- clamp + linear index (exact in f32) ----
            lim = [D - 1, H - 1, W - 1]
            c3 = []
            for k in range(3):
                ck = wpool.tile([128, COLS], dt.float32, tag=f"c{k}")
                nc.vector.tensor_scalar(
                    ck[:], r[k][:], 0.0, float(lim[k]), op0=Alu.max, op1=Alu.min
                )
                c3.append(ck)
            lin = wpool.tile([128, COLS], dt.float32, tag="lin")
            nc.vector.scalar_tensor_tensor(
                lin[:], c3[1][:], float(W), c3[2][:], op0=Alu.mult, op1=Alu.add
            )
            nc.vector.scalar_tensor_tensor(
                lin[:], c3[0][:], float(H * W), lin[:], op0=Alu.mult, op1=Alu.add
            )

            # ---- block id (floor(lin/256)) ----
            blockf = wpool.tile([128, COLS], dt.float32, tag="blockf")
            nc.vector.tensor_scalar(
                blockf[:], lin[:], 1.0 / BLK, FLOOR_C, op0=Alu.mult, op1=Alu.subtract
            )
            nc.vector.tensor_scalar(
                blockf[:], blockf[:], MAGIC, MAGIC, op0=Alu.add, op1=Alu.subtract
            )
            # ---- block ids -> wrapped-16 int16, replicated x8, via PE ----
            # wrap16[16g+q, 8c+u] = blockf[16u+q, c]:
            #   rhsW[p, 8c+u] = blockf[p, c] * (p//16 == u)   (maskw)
            #   out = selw.T @ rhsW with selw[p, m] = (p%16 == m%16)
            rhsW = wpool.tile([128, COLS, 8], dt.float32, tag="rhsW")
            nc.vector.tensor_tensor(
                rhsW[:],
                blockf[:]
                .rearrange("p (c one) -> p c one", one=1)
                .to_broadcast([128, COLS, 8]),
                maskw[:].rearrange("p (c u) -> p c u", u=8),
                op=Alu.mult,
            )
            psW = psXp.tile([128, SPTS], dt.float32, tag="psX")
            nc.tensor.matmul(
                psW[:, 0 : COLS * 8], selw[:], rhsW[:].rearrange("p c u -> p (c u)")
            )
            blk_idx = wpool.tile([128, COLS * 8], dt.int16, tag="blk_idx")
            nc.vector.tensor_copy(blk_idx[:], psW[:, 0 : COLS * 8])

            # ---- gathers: the only gpsimd work, back-to-back ----
            blocks_q = {}
            for s in range(NSLICE):
                blocks = bpool.tile([128, SCOLS, BLK], dt.bfloat16, tag="blocks")
                nc.gpsimd.dma_gather(
                    blocks[:],
                    atlas[:, :],
                    blk_idx[:, 64 * s : 64 * (s + 1)],
                    SPTS,
                    SPTS,
                    BLK,
                )
                blocks_q[s] = blocks

            # ---- bounds mask + in-block offset ----
            valid = wpool.tile([128, COLS], dt.float32, tag="valid")
            nc.vector.tensor_scalar(valid[:], r[0][:], 0.0, None, op0=Alu.is_ge)
            for k in range(3):
                if k > 0:
                    nc.vector.scalar_tensor_tensor(
                        valid[:], r[k][:], 0.0, valid[:], op0=Alu.is_ge, op1=Alu.mult
                    )
                nc.vector.scalar_tensor_tensor(
                    valid[:], r[k][:], float(lim[k]), valid[:],
                    op0=Alu.is_le, op1=Alu.mult,
                )
            off = wpool.tile([128, COLS], dt.float32, tag="off")
            nc.vector.scalar_tensor_tensor(
                off[:], blockf[:], float(-BLK), lin[:], op0=Alu.mult, op1=Alu.add
            )
            off_bf = wpool.tile([128, COLS], dt.bfloat16, tag="off_bf")
            nc.vector.tensor_copy(off_bf[:], off[:])

            # ---- offset one-hot for ALL cols in one op (hidden under
            #      gather 0): emask_all[p, c, o] = (iota256[o] == off[p, c])
            emask_all = wpool.tile([128, COLS, BLK], dt.bfloat16, tag="emask_all")
            nc.vector.tensor_tensor(
                emask_all[:],
                iota256[:]
                .rearrange("p (one e) -> p one e", one=1)
                .to_broadcast([128, COLS, BLK]),
                off_bf[:]
                .rearrange("p (c one) -> p c one", one=1)
                .to_broadcast([128, COLS, BLK]),
                op=Alu.is_equal,
            )

            # ---- per-slice: extract -> broadcast -> one-hot -> embed ----
            for s in range(NSLICE):
                blocks = blocks_q.pop(s)
                csl = slice(SCOLS * s, SCOLS * (s + 1))
                eprod = spool.tile([128, SCOLS, BLK], dt.bfloat16, tag="eprod")
                nc.vector.tensor_tensor(
                    eprod[:], blocks[:], emask_all[:, csl, :], op=Alu.mult
                )
                region = spool.tile([128, SCOLS], dt.float32, tag="region")
                nc.vector.tensor_reduce(
                    region[:],
                    eprod[:],
                    axis=mybir.AxisListType.X,
                    op=Alu.add,
                )
                nc.vector.tensor_tensor(
                    region[:], region[:], valid[:, csl], op=Alu.mult
                )

                # broadcast region over 117 partitions via masked matmul:
                # rhsB[p', (cc,p)] = region[p', cc] * (p' == p); psB = 1.T @ rhsB
                rhsB = spool.tile([128, SCOLS, 128], dt.bfloat16, tag="rhsB")
                nc.vector.tensor_tensor(
                    rhsB[:],
                    region[:]
                    .rearrange("p (c one) -> p c one", one=1)
                    .to_broadcast([128, SCOLS, 128]),
                    ident_f[:]
                    .rearrange("p (one q) -> p one q", one=1)
                    .to_broadcast([128, SCOLS, 128]),
                    op=Alu.mult,
                )
                psB_t = psXp.tile([128, SPTS], dt.float32, tag="psX")
                psB = psB_t[:][0 : REGION_MAX + 1, :]
                rb = rhsB[:].rearrange("p c q -> p (c q)")
                nc.tensor.matmul(psB[:, 0:512], ones_bf[:, :], rb[:, 0:512])
                nc.tensor.matmul(psB[:, 512:1024], ones_bf[:, :], rb[:, 512:1024])
                oh = ohpool.tile([REGION_MAX + 1, SPTS], dt.bfloat16, tag="oh")
                nc.vector.tensor_scalar(
                    oh[:], psB[:, :], iotar[0 : REGION_MAX + 1, :], None,
                    op0=Alu.is_equal,
                )

                for cc in range(SCOLS):
                    c = SCOLS * s + cc
                    lhs = oh[:, 128 * cc : 128 * (cc + 1)]
                    psO = psOp.tile([128, EMBED], dt.float32, tag="psO")
                    nc.tensor.matmul(psO[:, 0:512], lhs, table_bf[:, 0:512])
                    nc.tensor.matmul(psO[:, 512:768], lhs, table_bf[:, 512:768])
                    osb = opool.tile([128, EMBED], dt.bfloat16, tag="osb")
                    nc.scalar.copy(osb[:], psO[:])
                    nc.sync.dma_start(
                        out_v[:, EMBED * c : EMBED * (c + 1)], osb[:]
                    )

    nc.compile()
    return nc


def _consts():
    iotar = np.arange(128, dtype=np.float32).reshape(128, 1)
    ident = np.eye(128, dtype=np.float32)
    ones = np.ones((128, REGION_MAX + 1), dtype=np.float32)
    selw = np.zeros((128, 128), dtype=np.float32)
    for p in range(128):
        for m in range(128):
            if p % 16 == m % 16:
                selw[p, m] = 1.0
    maskw = np.zeros((128, COLS * 8), dtype=np.float32)
    for p in range(128):
        for c in range(COLS):
            maskw[p, 8 * c + (p // 16)] = 1.0
    iota256 = np.tile(np.arange(BLK, dtype=np.float32), (128, 1)).astype(np.float32)
    return iotar, ident, ones, selw, maskw, iota256


LAST_RESULTS = None


def kernel(patch_centers_voxels, mri_affine, aal_affine, aal_data, embed_table):
    global LAST_RESULTS
    import ml_dtypes
    from concourse.bass_utils import run_bass_kernel_spmd

    bf16 = ml_dtypes.bfloat16

    pc = np.asarray(patch_centers_voxels, dtype=np.float32)
    mri = np.asarray(mri_affine, dtype=np.float32)
    aal = np.asarray(aal_affine, dtype=np.float32)
    vol = np.asarray(aal_data, dtype=np.float32)
    tab = np.asarray(embed_table, dtype=np.float32)

    minv = np.linalg.inv(aal.astype(np.float32))
    M = (minv @ mri).astype(np.float32)
    m34 = M[:3, :]

    key = m34.tobytes()
    if key not in _cache:
        _cache[key] = _build(m34)
    nc = _cache[key]

    flat = vol.reshape(-1)
    atlas = np.zeros((NBLK * BLK,), dtype=np.float32)
    atlas[: flat.size] = flat
    atlas = atlas.reshape(NBLK, BLK).astype(bf16)

    tab_bf = tab.astype(bf16)

    iotar, ident, ones, selw, maskw, iota256 = _consts()

    shards = pc.reshape(NCORES, PTS, 3)
    in_maps = []
    for i in range(NCORES):
        in_maps.append(
            {
                "centers": np.ascontiguousarray(shards[i]),
                "atlas": atlas,
                "table": tab_bf,
                "iotar": iotar,
                "ident": ident,
                "ones": ones.astype(bf16),
                "selw": selw,
                "maskw": maskw,
                "iota256": iota256.astype(bf16),
            }
        )

    res = run_bass_kernel_spmd(nc, in_maps, core_ids=list(range(NCORES)))
    LAST_RESULTS = res
    out = np.concatenate(
        [np.asarray(res.results[i]["out"]).astype(np.float32) for i in range(NCORES)],
        axis=0,
    )
    return out.reshape(B, N, EMBED)
